# revision 1
# baseline (speedup 1.0000x reference)
"""Trainium2 Bass kernel for nn_DGCRNN (ChebConv K=3 GNN, robot-node output).

Math: the reference returns only node 0 (robot) of the ChebConv output, so
    out = r @ (W0 - W2 + v1[0]*W1 + 2*v2[0]*W2)
        + c1 @ W1 + c2 @ (2*W2) + cheb_b
with v1 = L_hat[0, :], v2 = (L_hat @ L_hat)[0, :] (host-computed from
edge_index), c1 = sum_i v1[i] * h_i, c2 = sum_i v2[i] * h_i over the 63
human-node embeddings h_i, and r the robot embedding.

Sharding: pure data parallel over the batch dim (512 / 8 cores = 64 each);
all weights and graph data replicated.

Implementation: raw bacc (manual semaphores, no Tile) for minimal overhead.
Feature-major layout; MLP layer-1 biases folded into the matmuls via an
appended ones-row (K augmentation); wh2 duplicated along M so h2 lands twice
on 128 partitions, letting one tensor_tensor multiply by the stacked [v1;v2]
pattern and one segmented reduce produce c1,c2 stacked on 128 partitions;
the final ChebConv combine is 3 accumulating matmuls (incl. bias as a rank-1
ones x cheb_b outer product). The big MLP matmuls use float32r (TF32-rate,
single pass); everything else stays fp32.
"""

import numpy as np

B, N, F, HID = 512, 64, 64, 128
ROBOT_DIM, HUMAN_DIM = 9, 5
NCORES = 8
BL = B // NCORES      # 64 batches per core
NH = N - 1            # 63 human nodes
TOK = BL * NH         # 4032 human tokens per core
SL = 504              # tokens per matmul slice (one PSUM bank)
NPAIR = 4             # pipeline pairs; pair = 2 slices = 1008 tokens = 16 batches
PBATCH = 16           # batches per pair

_STATE = {}


def _build_bass():
    import os

    import concourse.bass as bass
    from concourse import bacc, mybir

    f32 = mybir.dt.float32
    f32r = mybir.dt.float32 if os.environ.get("DGCRNN_NO_F32R") else mybir.dt.float32r
    AF = mybir.ActivationFunctionType
    ALU = mybir.AluOpType
    AX = mybir.AxisListType

    nc = bacc.Bacc("TRN2", target_bir_lowering=False, debug=False)

    # --- DRAM I/O ---
    d_hTa = nc.dram_tensor("hTa", [HUMAN_DIM + 1, TOK], f32r, kind="ExternalInput").ap()
    d_pa = nc.dram_tensor("pa", [HID, 257], f32, kind="ExternalInput").ap()
    d_pb = nc.dram_tensor("pb", [HID, 320], f32, kind="ExternalInput").ap()
    d_pr = nc.dram_tensor("pr", [HID, 256], f32r, kind="ExternalInput").ap()
    d_out = nc.dram_tensor("out", [BL, F], f32, kind="ExternalOutput").ap()

    # --- SBUF ---
    hTa = nc.alloc_sbuf_tensor("hTa_sb", [HUMAN_DIM + 1, TOK], f32r).ap()
    pa = nc.alloc_sbuf_tensor("pa_sb", [HID, 257], f32).ap()
    pb = nc.alloc_sbuf_tensor("pb_sb", [HID, 320], f32).ap()
    pr_ = nc.alloc_sbuf_tensor("pr_sb", [HID, 256], f32r).ap()
    h1 = nc.alloc_sbuf_tensor("h1_sb", [HID, TOK], f32r).ap()
    h2d = nc.alloc_sbuf_tensor("h2d_sb", [2 * F, BL, NH], f32).ap()
    tmp = nc.alloc_sbuf_tensor("tmp_sb", [2 * F, BL, NH], f32).ap()
    c12 = nc.alloc_sbuf_tensor("c12_sb", [2 * F, BL], f32).ap()
    r1 = nc.alloc_sbuf_tensor("r1_sb", [HID, BL], f32).ap()
    r2 = nc.alloc_sbuf_tensor("r2_sb", [F, BL], f32).ap()
    out_sb = nc.alloc_sbuf_tensor("out_sb", [BL, F], f32).ap()

    # pack_a slices (robot path)
    wr2 = pa[:, 0:64]
    br2 = pa[0:64, 64:65]
    rTa = pa[0:ROBOT_DIM + 1, 65:129]
    wr1a = pa[0:ROBOT_DIM + 1, 129:257]
    # pack_b slices (c-sum + final combine)
    W12 = pb[:, 0:64]
    v12 = pb[:, 64:127]
    bh2d = pb[:, 127:128]
    Ar = pb[0:64, 128:192]
    onesr = pb[0:1, 192:256]
    chebb = pb[0:1, 256:320]
    # packr slices (f32r)
    wh2d = pr_[:, 0:128]
    wh1a = pr_[0:HUMAN_DIM + 1, 128:256]

    # --- PSUM: ping-pong, 8 banks total ---
    ph1 = nc.alloc_psum_tensor("ph1", [HID, 2048], f32).ap()      # 4 banks
    ph2 = nc.alloc_psum_tensor("ph2", [2 * F, 2048], f32).ap()    # 4 banks
    pr1 = ph1[:, 0:BL]          # robot L1 out, freed by r1relu before L1(0)
    pr2 = ph2[:F, 0:BL]         # robot L2 out, freed by r2relu before L2(0)
    po = ph2[:BL, 0:F]          # final out, after relu2(2) consumed ping

    v12_b = bass.AP(v12.tensor, v12.offset, [list(v12.ap[0]), [0, PBATCH], [1, NH]])

    # --- semaphores ---
    sdh = [nc.alloc_semaphore(f"sdh{c}") for c in range(NPAIR)]  # per hTa chunk
    sdr = nc.alloc_semaphore("sdr")    # pack_a DMA
    sdw = nc.alloc_semaphore("sdw")    # packr DMA
    sdf = nc.alloc_semaphore("sdf")    # pack_b DMA
    sp = nc.alloc_semaphore("sp")      # PE groups done
    sa = nc.alloc_semaphore("sa")      # ACT ops done
    sv = nc.alloc_semaphore("sv")      # DVE ops done
    sg = nc.alloc_semaphore("sg")      # GPS ops done
    # sq is inc-only (out-DMA completion is guaranteed by the end-of-block
    # drain); it is deliberately NOT cleared -- nothing ever waits on it
    sq = nc.alloc_semaphore("sq")
    all_sems = sdh + [sdr, sdw, sdf, sp, sa, sv, sg]

    PC = 1008  # tokens per pair

    def ping(p):
        return (p % 2) * 1024

    def pair_b(p):
        return slice(p * PBATCH, (p + 1) * PBATCH)

    def ph_in(ph, p):
        o = ping(p)
        return bass.AP(ph.tensor, ph.offset + o, [list(ph.ap[0]), [512, 2], [1, SL]])

    with nc.Block(no_gpsimd_drain=True) as block:

        @block.sync
        def _(sync):
            for c in range(NPAIR):
                sync.dma_start(
                    out=hTa[:, c * PC : (c + 1) * PC],
                    in_=d_hTa[:, c * PC : (c + 1) * PC],
                ).then_inc(sdh[c], 16)
            sync.wait_ge(sv, 8)
            sync.dma_start(out=d_out[:], in_=out_sb[:]).then_inc(sq, 16)

        @block.tensor
        def _(tensor):
            # sp: 1=rMM1 2=rMM2 3=L1(0) 4=L1(1) 5=L2(0) 6=L1(2) 7=L2(1)
            #     8=L1(3) 9=L2(2) 10=L2(3) 11=finals
            def l1(p, *waits):
                for s, v in waits:
                    tensor.wait_ge(s, v)
                o = ping(p)
                tensor.matmul(ph1[:, o : o + SL], wh1a, hTa[:, p * PC : p * PC + SL],
                              start=True, stop=True)
                tensor.matmul(ph1[:, o + 512 : o + 512 + SL], wh1a,
                              hTa[:, p * PC + SL : (p + 1) * PC],
                              start=True, stop=True).then_inc(sp)

            def l2(p, *waits):
                for s, v in waits:
                    tensor.wait_ge(s, v)
                o = ping(p)
                tensor.matmul(ph2[:, o : o + SL], wh2d, h1[:, p * PC : p * PC + SL],
                              start=True, stop=True)
                tensor.matmul(ph2[:, o + 512 : o + 512 + SL], wh2d,
                              h1[:, p * PC + SL : (p + 1) * PC],
                              start=True, stop=True).then_inc(sp)

            tensor.wait_ge(sdr, 16)
            tensor.matmul(pr1, wr1a, rTa, start=True, stop=True).then_inc(sp)   # 1
            tensor.wait_ge(sa, 1)
            tensor.matmul(pr2, wr2, r1[:], start=True, stop=True).then_inc(sp)  # 2
            tensor.wait_ge(sdw, 16)
            l1(0, (sdh[0], 16), (sa, 1))            # 3 (pr1 region WAR)
            l1(1, (sdh[1], 16))                     # 4
            l2(0, (sa, 3))                          # 5 (relu1(0) + r2relu WAR)
            l1(2, (sdh[2], 16), (sa, 3))            # 6 (ping freed)
            l2(1, (sa, 4))                          # 7
            l1(3, (sdh[3], 16), (sa, 4))            # 8
            l2(2, (sa, 6))                          # 9 (ping WAR via relu2(0)<=6)
            l2(3, (sa, 8))                          # 10
            tensor.wait_ge(sv, 7)                   # c12 ready
            tensor.wait_ge(sa, 9)                   # po region WAR (relu2(2))
            tensor.wait_ge(sdf, 16)
            tensor.matmul(po, c12[:], W12, start=True, stop=False)
            tensor.matmul(po, r2[:], Ar, start=False, stop=False)
            tensor.matmul(po, onesr, chebb, start=False, stop=True).then_inc(sp)  # 11

        @block.scalar
        def _(scalar):
            scalar.dma_start(out=pa[:], in_=d_pa[:]).then_inc(sdr, 16)
            scalar.dma_start(out=pr_[:], in_=d_pr[:]).then_inc(sdw, 16)
            scalar.dma_start(out=pb[:], in_=d_pb[:]).then_inc(sdf, 16)
            # sa: 1=r1relu 2=r2relu 3=relu1(0) 4=relu1(1) 5=relu2(0)
            #     6=relu1(2) 7=relu2(1) 8=relu1(3)
            scalar.wait_ge(sp, 1)
            scalar.activation(r1[:], pr1, AF.Relu).then_inc(sa)
            scalar.wait_ge(sp, 2)
            scalar.activation(r2[:], pr2, AF.Relu, bias=br2).then_inc(sa)
            scalar.wait_ge(sp, 3)
            scalar.activation(h1[:, 0:PC], ph_in(ph1, 0), AF.Relu).then_inc(sa)
            scalar.wait_ge(sp, 4)
            scalar.activation(h1[:, PC : 2 * PC], ph_in(ph1, 1), AF.Relu).then_inc(sa)
            scalar.wait_ge(sp, 5)
            scalar.wait_ge(sdf, 16)
            scalar.activation(h2d[:, pair_b(0), :], ph_in(ph2, 0), AF.Relu,
                              bias=bh2d).then_inc(sa)
            scalar.wait_ge(sp, 6)
            scalar.activation(h1[:, 2 * PC : 3 * PC], ph_in(ph1, 2), AF.Relu).then_inc(sa)
            scalar.wait_ge(sp, 7)
            scalar.activation(h2d[:, pair_b(1), :], ph_in(ph2, 1), AF.Relu,
                              bias=bh2d).then_inc(sa)
            scalar.wait_ge(sp, 8)
            scalar.activation(h1[:, 3 * PC : 4 * PC], ph_in(ph1, 3), AF.Relu).then_inc(sa)
            scalar.wait_ge(sp, 9)
            scalar.activation(h2d[:, pair_b(2), :], ph_in(ph2, 2), AF.Relu,
                              bias=bh2d).then_inc(sa)
            scalar.wait_ge(sp, 10)
            scalar.activation(h2d[:, pair_b(3), :], ph_in(ph2, 3), AF.Relu,
                              bias=bh2d).then_inc(sa)

        @block.vector
        def _(vector):
            # sv: 1=mul(0) 2=red(0) 3=mul(1) 4=red(1) 5=mul(3) 6=red(3)
            #     7=red(2) 8=copy   (mul(2) on GPS)
            def red(p, *waits):
                for s, v in waits:
                    vector.wait_ge(s, v)
                vector.tensor_reduce(c12[:, pair_b(p)], tmp[:, pair_b(p), :],
                                     axis=AX.X, op=ALU.add).then_inc(sv)

            vector.wait_ge(sa, 5)
            vector.wait_ge(sdf, 16)
            vector.tensor_tensor(tmp[:, pair_b(0), :], h2d[:, pair_b(0), :],
                                 v12_b, ALU.mult).then_inc(sv)
            red(0, (sv, 1))
            vector.wait_ge(sa, 7)
            vector.tensor_tensor(tmp[:, pair_b(1), :], h2d[:, pair_b(1), :],
                                 v12_b, ALU.mult).then_inc(sv)
            red(1, (sv, 3))
            vector.wait_ge(sa, 10)
            vector.tensor_tensor(tmp[:, pair_b(3), :], h2d[:, pair_b(3), :],
                                 v12_b, ALU.mult).then_inc(sv)
            red(3, (sv, 5))
            red(2, (sg, 1))
            vector.wait_ge(sp, 11)
            vector.tensor_copy(out_sb[:], po).then_inc(sv)

        @block.gpsimd
        def _(gpsimd):
            gpsimd.wait_ge(sa, 9)
            gpsimd.tensor_tensor(tmp[:, pair_b(2), :], h2d[:, pair_b(2), :],
                                 v12_b, ALU.mult).then_inc(sg)

    # sems must return to 0 for NEFF re-execution; the Block exit emitted an
    # all-engine barrier, so clearing here is safe.
    nc.clear_and_free_semaphores(all_sems)

    nc.compile()
    return nc


def _host_prep(robot_x, human_x, edge_index, wr1_w, wr1_b, wr2_w, wr2_b,
               wh1_w, wh1_b, wh2_w, wh2_b, cheb_w, cheb_b):
    """Compute graph vectors + packed weights on host; build per-core inputs."""
    robot_x = np.ascontiguousarray(np.asarray(robot_x, dtype=np.float32))
    human_x = np.ascontiguousarray(np.asarray(human_x, dtype=np.float32))
    ei = np.asarray(edge_index)
    src, dst = ei[0].astype(np.int64), ei[1].astype(np.int64)

    f32 = np.float32
    deg = np.zeros(N, f32)
    np.add.at(deg, src, f32(1.0))
    dinv = np.where(deg > 0, deg.astype(f32) ** f32(-0.5), f32(0.0)).astype(f32)
    w = -(dinv[src] * dinv[dst])
    L = np.zeros((N, N), f32)
    np.add.at(L, (dst, src), w)
    v1 = L[0].astype(f32)
    v2 = (v1 @ L).astype(f32)

    W0, W1, W2 = (np.asarray(cheb_w, f32)[k] for k in range(3))
    wh1_w = np.asarray(wh1_w, f32); wh1_b = np.asarray(wh1_b, f32)
    wh2_w = np.asarray(wh2_w, f32); wh2_b = np.asarray(wh2_b, f32)
    wr1_w = np.asarray(wr1_w, f32); wr1_b = np.asarray(wr1_b, f32)
    wr2_w = np.asarray(wr2_w, f32); wr2_b = np.asarray(wr2_b, f32)
    cheb_b = np.asarray(cheb_b, f32)

    pa = np.zeros((HID, 257), f32)
    pa[:, 0:64] = wr2_w
    pa[0:64, 64] = wr2_b
    pa[0:ROBOT_DIM, 129:257] = wr1_w
    pa[ROBOT_DIM, 129:257] = wr1_b
    pb = np.zeros((HID, 320), f32)
    pb[0:64, 0:64] = W1
    pb[64:128, 0:64] = f32(2.0) * W2
    pb[0:64, 64:127] = np.tile(v1[1:], (F, 1))
    pb[64:128, 64:127] = np.tile(v2[1:], (F, 1))
    pb[0:64, 127] = wh2_b
    pb[64:128, 127] = wh2_b
    pb[0:64, 128:192] = W0 - W2 + v1[0] * W1 + f32(2.0) * v2[0] * W2
    pb[0, 192:256] = f32(1.0)
    pb[0, 256:320] = cheb_b
    pr = np.zeros((HID, 256), f32)
    pr[:, 0:128] = np.hstack([wh2_w, wh2_w])
    pr[0:HUMAN_DIM, 128:256] = wh1_w
    pr[HUMAN_DIM, 128:256] = wh1_b
    shared = {"pa": pa, "pb": pb, "pr": pr}

    in_maps = []
    ones_tok = np.ones((1, TOK), f32)
    for c in range(NCORES):
        bs = slice(c * BL, (c + 1) * BL)
        hT = human_x[bs].transpose(2, 0, 1).reshape(HUMAN_DIM, TOK)
        m = dict(shared)
        m["hTa"] = np.ascontiguousarray(np.vstack([hT, ones_tok]))
        pac = shared["pa"].copy()
        pac[0:ROBOT_DIM, 65:129] = robot_x[bs, 0, :].T
        pac[ROBOT_DIM, 65:129] = f32(1.0)
        m["pa"] = pac
        in_maps.append(m)
    return in_maps


def run(inputs, trace=False, tmpdir=None):
    """Run the Bass kernel on 8 cores. Returns (full_output, BassKernelResults)."""
    from concourse.bass_utils import run_bass_kernel_spmd

    if "nc" not in _STATE:
        _STATE["nc"] = _build_bass()
    nc = _STATE["nc"]

    in_maps = _host_prep(**inputs)
    res = run_bass_kernel_spmd(
        nc, in_maps, list(range(NCORES)), trace=trace, tmpdir=tmpdir
    )
    out = np.concatenate([res.results[c]["out"] for c in range(NCORES)], axis=0)
    return out, res


def kernel(**inputs) -> np.ndarray:
    out, _ = run(inputs, trace=False)
    return out

